# revision 15
# baseline (speedup 1.0000x reference)
"""DiffusionNetBlock on 8 Trainium2 NeuronCores.

Strategy (data-parallel over batch x row-halves, 8 cores = 4 batches x 2):
  core c = 2*b + h owns batch b and half of its mesh vertices.

Host-side prep (sharding/layout only, no model math beyond input folding):
  - fold vertex_areas into x_in, precompute the spectral heat scale
    exp(-evals x times) (tiny [K,P] per batch), transpose weights.
  - the sparse gradient (COO, E=160k edges/batch) is laid out for the
    device: rows of each batch are degree-sorted into 128-row blocks,
    blocks dealt to the two cores, and each block padded to a fixed
    per-slot degree D (equalized across cores so one NEFF serves all 8).
    Edges become dense fp8 streams xev = evecs[col] tiled as DoubleRow
    pairs [128 edges, 2 k-tiles, K]; the weighted segment-sum over rows
    is a 256-deep fp8 DoubleRow matmul with a selector carrying the
    gradX/gradY values, fully on the PE with f32 PSUM accumulation.

Device kernel (Bass/Tile, same program on all 8 cores):
  A: x_spec = evecs^T @ (a*x_in)          (PSUM accum over 157 chunks)
     s2 = exp(-lam t) * x_spec            (one DVE op)
  B: x_diffuse^T = s2^T @ evecs^T         (kept in SBUF, [P, rows])
  C: agX^T/agY^T per 128-row block via fp8 DoubleRow selector matmuls
     gx^T = s2^T @ agX^T, gy^T = s2^T @ agY^T
  D: xg = tanh(gx*(B_re gx) + gy*(B_im gy))
  E: 3-layer MLP on [x_in; x_diffuse; xg], + residual
  All of C-E runs in transposed [feature, row] layout in 512-row groups.
Host inverse-permutes/transposes the output.
"""

import math
import os
import sys

import ml_dtypes
import numpy as np

sys.path.insert(0, "/opt/trn_rl_repo")

from concourse import bass, mybir  # noqa: E402
from concourse import bass_utils  # noqa: E402
from concourse.tile import TileContext  # noqa: E402
from concourse.vector_clock import ScopedClock, VectorClock  # noqa: E402

B, N, P, K, E = 4, 20000, 128, 128, 160000
NCORES = 8
NBLK = 79                    # 128-row blocks per core
ROWS = NBLK * 128            # 10112 row slots per core
TOTBLK = 2 * NBLK            # 158 blocks per batch (20224 >= 20000 row slots)
GRP = 4                      # blocks per 512-wide processing group
NCHUNK = (N + 127) // 128    # 157 n-chunks for phase A (20096 padded)
NPAD = NCHUNK * 128

f32 = mybir.dt.float32
f32r = mybir.dt.float32r
f16 = mybir.dt.float16
f8 = mybir.dt.float8e4
f8np = ml_dtypes.float8_e4m3


# --------------------------------------------------------------- BIR fixup
# This toolchain's walrus encodes at most ONE sync wait per instruction
# ("Too many sync wait commands"), but Tile's add_semaphores freely
# attaches several. Hoist excess waits onto EventSemaphore carriers on
# the same engine, inserted just before the over-subscribed instruction.

def _split_excess_waits(bir_json: bytes) -> bytes:
    import json
    d = json.loads(bir_json)
    n_split = 0
    for fn in d.get("functions", []):
        for blk in fn.get("blocks", []):
            insts = blk.get("instructions")
            if not insts:
                continue
            out = []
            changed = False
            for ins in insts:
                si = ins.get("sync_info") or {}
                ow = si.get("on_wait") or []
                if len(ow) > 1 and "engine" in ins:
                    for w in ow[:-1]:
                        n_split += 1
                        out.append({
                            "debug": ins.get("debug", 0),
                            "engine": ins["engine"],
                            "ins": [],
                            "outs": [],
                            "name": f"{ins['name']}-xw{n_split}",
                            "opcode": "EventSemaphore",
                            "sync_info": {"on_update": [], "on_wait": [w]},
                        })
                    si["on_wait"] = [ow[-1]]
                    changed = True
                out.append(ins)
            if changed:
                blk["instructions"] = out
    if n_split == 0:
        return bir_json
    return json.dumps(d).encode()


_orig_compile_bir_kernel = bass_utils.compile_bir_kernel


def _patched_compile_bir_kernel(bir_json, tmpdir, neff_name="file.neff"):
    return _orig_compile_bir_kernel(_split_excess_waits(bir_json), tmpdir,
                                    neff_name)


def _install_birfix():
    from concourse import bass2jax
    if bass_utils.compile_bir_kernel.__name__ != "_patched_compile_bir_kernel":
        bass_utils.compile_bir_kernel = _patched_compile_bir_kernel
    if bass2jax.compile_bir_kernel.__name__ != "_patched_compile_bir_kernel":
        bass2jax.compile_bir_kernel = _patched_compile_bir_kernel


_install_birfix()


class FixedTileContext(TileContext):
    """Stock _drain_and_barrier stuffs every outstanding sem wait onto one
    SP Drain; TRN2 TPB_CTRL encoding only fits 1-2 sync waits and walrus
    dies with "Too many sync wait commands". Split the final global-clock
    wait into one Drain per logical proc."""

    def _drain_and_barrier(self, tick_clock, wait_clock):
        gc = tick_clock.global_clock
        n = len(gc)
        for p in range(n):
            if gc[p] > 0:
                vec = [0] * n
                vec[p] = gc[p]
                w = self.nc.sync.drain()
                wait_clock.add_sem_waits(w.ins, ScopedClock({None: VectorClock(vec)}))
        # The per-proc drains above run serially on SP, so every wait is
        # already satisfied here; emit the final drain bare.
        self.nc.sync.drain()
        self.nc.all_engine_barrier()
        assert self.sems is not None
        popped = self.nc._tile_sem_poison_stack.pop()
        assert popped is self._sem_poison
        self.nc.clear_and_free_semaphores(list(self.sems.allocated().values()))
        self.nc.all_engine_barrier()


# ---------------------------------------------------------------- host prep


def _plan_slots(grad_rows):
    """Degree-sort rows per batch into blocks, deal to cores, and compute
    the global per-slot degree D (equalized across all 8 cores)."""
    perms = []          # per batch: [TOTBLK*128] row ids (-1 = pad)
    degs = []
    d_blocks = np.zeros((B, 2, NBLK), np.int64)
    for b in range(B):
        deg = np.bincount(np.asarray(grad_rows[b]), minlength=N)
        order = np.argsort(-deg, kind="stable")
        perm = np.concatenate([order, np.full(TOTBLK * 128 - N, -1, np.int64)])
        dblk = deg[np.maximum(perm, 0)] * (perm >= 0)
        dblk = dblk.reshape(TOTBLK, 128).max(axis=1)
        for i in range(TOTBLK):
            d_blocks[b, i % 2, i // 2] = dblk[i]
        perms.append(perm)
        degs.append(deg)
    d_slots = np.maximum(d_blocks.max(axis=(0, 1)), 1)   # [NBLK]
    assert d_slots.max() <= 128, d_slots.max()
    return perms, degs, d_slots


def _slot_geometry(d_slots):
    """fp8 DoubleRow geometry. Per slot: half-degree dh (edges per row per
    k-tile), rows-per-instruction R, instructions T, stream tile offset,
    and the column offset of this slot's selector block (4*R*T cols: per
    tile, per k-tile half, [selX_R | selY_R])."""
    geo = []
    t_off = 0
    s_off = 0
    for D in d_slots.tolist():
        dh = (D + 1) // 2
        R = 128 // dh
        T = math.ceil(128 / R)
        geo.append((dh, R, T, t_off, s_off))
        t_off += T
        s_off += 4 * T * R
    return geo, t_off, s_off


def build_host_data(inputs):
    x_in = np.asarray(inputs["x_in"], np.float32)
    areas = np.asarray(inputs["vertex_areas"], np.float32)
    evals = np.asarray(inputs["evals"], np.float32)
    evecs = np.asarray(inputs["evecs"], np.float32)
    gxv = np.asarray(inputs["gradX_vals"], np.float32)
    gyv = np.asarray(inputs["gradY_vals"], np.float32)
    grows = np.asarray(inputs["grad_rows"], np.int64)
    gcols = np.asarray(inputs["grad_cols"], np.int64)
    times = np.clip(np.asarray(inputs["diffusion_times"], np.float32), 1e-8, None)
    W1 = np.asarray(inputs["W1"], np.float32)
    b1 = np.asarray(inputs["b1"], np.float32)
    W2 = np.asarray(inputs["W2"], np.float32)
    b2 = np.asarray(inputs["b2"], np.float32)
    W3 = np.asarray(inputs["W3"], np.float32)
    b3 = np.asarray(inputs["b3"], np.float32)
    B_re = np.asarray(inputs["B_re"], np.float32)
    B_im = np.asarray(inputs["B_im"], np.float32)

    perms, degs, d_slots = _plan_slots(grows)
    geo, TT, SELTOT = _slot_geometry(d_slots)

    # phase A inputs, partition-major: ax[p, c, 0, :] = evecs row c*128+p,
    # ax[p, c, 1, :] = (a*x_in) row c*128+p
    ax_all = np.zeros((B, NPAD, 2, P), np.float16)
    ax_all[:, :N, 0, :] = evecs.astype(np.float16)
    ax_all[:, :N, 1, :] = (x_in * areas[:, :, None]).astype(np.float16)
    ax_all = np.ascontiguousarray(
        ax_all.reshape(B, NCHUNK, 128, 2, P).transpose(0, 2, 1, 3, 4))

    in_maps = []
    core_perm = []
    for b in range(B):
        rows_b, cols_b = grows[b], gcols[b]
        esort = np.argsort(rows_b, kind="stable")
        deg = degs[b]
        rowptr = np.zeros(N + 1, np.int64)
        rowptr[1:] = np.cumsum(deg)
        scale = np.exp(-evals[b][:, None] * times[None, :]).astype(np.float32)
        ev8 = evecs[b].astype(f8np)
        for h in range(2):
            blk_ids = 2 * np.arange(NBLK) + h          # block index within batch
            perm_own = perms[b].reshape(TOTBLK, 128)[blk_ids].reshape(-1)  # [ROWS]
            core_perm.append(perm_own)
            pv = np.maximum(perm_own, 0)
            valid = perm_own >= 0

            # per-row padded edge grid, slot by slot, split into 2 k-tile
            # halves for fp8 DoubleRow (256-edge contraction / instruction)
            col_stream = np.zeros((TT, 2, 128), np.int64)
            selxy = np.zeros((128, SELTOT), f8np)
            for s, (dh, R, T, toff, soff) in enumerate(geo):
                rows_blk = perm_own[s * 128:(s + 1) * 128]
                rb = np.maximum(rows_blk, 0)
                cnt = np.where(rows_blk >= 0, deg[rb], 0)
                c0 = (cnt + 1) // 2                      # ceil half
                starts = [np.zeros_like(c0), c0]
                halves = [c0, cnt - c0]
                G = T * R
                e = np.arange(128)
                ei, ed = e // dh, e % dh             # row-in-tile, slot-in-row
                emask = ei < R
                eis = np.where(emask, ei, 0)
                sel_t = np.zeros((T, 2, 128, 2 * R), np.float32)
                for hh in range(2):
                    idx = (rowptr[rb][:, None] + starts[hh][:, None]
                           + np.arange(dh)[None, :])
                    mask = np.arange(dh)[None, :] < halves[hh][:, None]
                    eid = esort[np.where(mask, idx, 0)]
                    cm = np.where(mask, cols_b[eid], 0)          # [128, dh]
                    vxm = np.where(mask, gxv[b][eid], 0.0)
                    vym = np.where(mask, gyv[b][eid], 0.0)
                    pad = ((0, G - 128), (0, 0))
                    cmp_ = np.pad(cm, pad).reshape(T, R * dh)
                    col_stream[toff:toff + T, hh] = np.pad(
                        cmp_, ((0, 0), (0, 128 - R * dh)))
                    vxm = np.pad(vxm, pad).reshape(T, R, dh)
                    vym = np.pad(vym, pad).reshape(T, R, dh)
                    sel_t[:, hh, e, eis] = vxm[:, eis, ed] * emask
                    sel_t[:, hh, e, R + eis] = vym[:, eis, ed] * emask
                selxy[:, soff:soff + 4 * T * R] = (
                    sel_t.transpose(2, 0, 1, 3).reshape(128, T * 4 * R)
                    .astype(f8np))

            evg = np.ascontiguousarray(
                ev8[col_stream].transpose(2, 0, 1, 3))   # [128, TT, 2, K]

            in_maps.append({
                "evg": evg,
                "selxy": selxy,
                "ax": ax_all[b],
                "evsT": np.ascontiguousarray(
                    (evecs[b][pv].T * valid[None, :]).astype(np.float16)),
                "xinT": np.ascontiguousarray(
                    (x_in[b][pv].T * valid[None, :]).astype(np.float16)),
                "scale": scale,
                "w1t": np.ascontiguousarray(W1.T.reshape(3, P, P).astype(np.float16)),
                "w2t": np.ascontiguousarray(W2.T.astype(np.float16)),
                "w3t": np.ascontiguousarray(W3.T.astype(np.float16)),
                "bret": np.ascontiguousarray(B_re.T.astype(np.float16)),
                "bimt": np.ascontiguousarray(B_im.T.astype(np.float16)),
                "b1": b1.reshape(P, 1).copy(),
                "b2": b2.reshape(P, 1).copy(),
                "b3": b3.reshape(P, 1).copy(),
            })

    meta = {"geo": geo, "TT": TT, "SELTOT": SELTOT, "d_slots": d_slots}
    return in_maps, core_perm, meta


# ------------------------------------------------------------ device kernel


def build_bass(meta):
    geo = meta["geo"]
    TT = meta["TT"]
    SELTOT = meta["SELTOT"]

    nc = bass.Bass("TRN2", target_bir_lowering=False, debug=False,
                   num_devices=NCORES)

    evg_d = nc.dram_tensor("evg", [128, TT, 2, K], f8, kind="ExternalInput")
    selxy_d = nc.dram_tensor("selxy", [128, SELTOT], f8, kind="ExternalInput")
    ax_d = nc.dram_tensor("ax", [128, NCHUNK, 2, P], f16, kind="ExternalInput")
    evsT_d = nc.dram_tensor("evsT", [K, ROWS], f16, kind="ExternalInput")
    xinT_d = nc.dram_tensor("xinT", [P, ROWS], f16, kind="ExternalInput")
    scale_d = nc.dram_tensor("scale", [K, P], f32, kind="ExternalInput")
    w1t_d = nc.dram_tensor("w1t", [3, P, P], f16, kind="ExternalInput")
    w2t_d = nc.dram_tensor("w2t", [P, P], f16, kind="ExternalInput")
    w3t_d = nc.dram_tensor("w3t", [P, P], f16, kind="ExternalInput")
    bret_d = nc.dram_tensor("bret", [P, P], f16, kind="ExternalInput")
    bimt_d = nc.dram_tensor("bimt", [P, P], f16, kind="ExternalInput")
    b1_d = nc.dram_tensor("b1", [P, 1], f32, kind="ExternalInput")
    b2_d = nc.dram_tensor("b2", [P, 1], f32, kind="ExternalInput")
    b3_d = nc.dram_tensor("b3", [P, 1], f32, kind="ExternalInput")
    outT_d = nc.dram_tensor("outT", [P, ROWS], f16, kind="ExternalOutput")

    AF = mybir.ActivationFunctionType
    DR = mybir.MatmulPerfMode.DoubleRow
    XCH = 16       # evg DoubleRow tiles per DMA chunk
    EVG_BUFS = 8   # evg ring depth (chunks)

    with FixedTileContext(nc) as tc:
        with (
            tc.tile_pool(name="consts", bufs=1) as cpool,
            tc.tile_pool(name="xdpool", bufs=1) as xdpool,
            tc.tile_pool(name="pX", bufs=EVG_BUFS) as pX,
        ):
            scale_t = cpool.tile([K, P], f32, tag="scale")
            nc.scalar.dma_start(scale_t[:], scale_d[:])
            wh = cpool.tile([P, 7, P], f16, tag="wh")
            nc.scalar.dma_start(wh[:, 0:3, :], w1t_d[:].rearrange("s p q -> p s q"))
            nc.scalar.dma_start(wh[:, 3, :], w2t_d[:])
            nc.scalar.dma_start(wh[:, 4, :], w3t_d[:])
            nc.scalar.dma_start(wh[:, 5, :], bret_d[:])
            nc.scalar.dma_start(wh[:, 6, :], bimt_d[:])
            w1t_t = wh[:, 0:3, :]
            w2t_t = wh[:, 3, :]
            w3t_t = wh[:, 4, :]
            bret_t = wh[:, 5, :]
            bimt_t = wh[:, 6, :]
            b1_t = cpool.tile([P, 1], f32, tag="b1")
            nc.scalar.dma_start(b1_t[:], b1_d[:])
            b2_t = cpool.tile([P, 1], f32, tag="b2")
            nc.scalar.dma_start(b2_t[:], b2_d[:])
            b3_t = cpool.tile([P, 1], f32, tag="b3")
            nc.scalar.dma_start(b3_t[:], b3_d[:])
            s2_t = cpool.tile([K, P], f32, tag="s2")
            s2h_t = cpool.tile([K, P], f16, tag="s2h")
            xdT_t = xdpool.tile([P, ROWS], f16, tag="xdT")
            xinT_t = xdpool.tile([P, ROWS], f16, tag="xinT")

            # eager evg prefetch: the whole stream is an input, so issue
            # every chunk DMA up front on gpsimd (which carries nothing
            # else — the ring's WAR waits stall only this queue) and let
            # the pool's ring recycling pace it against consumption.
            evg_chunks = []
            for t0 in range(0, TT, XCH):
                w = min(XCH, TT - t0)
                xt = pX.tile([128, XCH, 2, K], f8, tag="evg")
                nc.gpsimd.dma_start(xt[:, :w], evg_d[:, t0:t0 + w])
                evg_chunks.append(xt)

            def evg_tile(t):
                return evg_chunks[t // XCH][:, t % XCH, :, :]

            # ---------------- phase A: x_spec, s2
            ACH = 8
            with (
                tc.tile_pool(name="pA", bufs=6) as pA,
                tc.tile_pool(name="psA", bufs=1, space="PSUM") as psA_pool,
            ):
                psA = psA_pool.tile([K, P], f32, tag="psA")
                for c0 in range(0, NCHUNK, ACH):
                    w = min(ACH, NCHUNK - c0)
                    ax_t = pA.tile([128, ACH, 2, P], f16, tag="axA")
                    nc.sync.dma_start(ax_t[:, :w], ax_d[:, c0:c0 + w])
                    for i in range(w):
                        nc.tensor.matmul(
                            psA[:], ax_t[:, i, 0, :], ax_t[:, i, 1, :],
                            start=(c0 + i == 0), stop=(c0 + i == NCHUNK - 1),
                        )
                nc.vector.tensor_mul(s2_t[:], scale_t[:], psA[:])
                nc.vector.tensor_copy(s2h_t[:], s2_t[:])

            # ---------------- phase B: x_diffuse^T resident in SBUF
            with (
                tc.tile_pool(name="pB", bufs=10) as pB,
                tc.tile_pool(name="psB", bufs=2, space="PSUM") as psB_pool,
            ):
                for g0 in range(0, ROWS, 512):
                    w = min(512, ROWS - g0)
                    evsT_t = pB.tile([K, 512], f16, tag="evsTB")
                    nc.sync.dma_start(evsT_t[:, :w], evsT_d[:, g0:g0 + w])
                    psB = psB_pool.tile([P, 512], f32, tag="psB")
                    nc.tensor.matmul(
                        psB[:, :w], s2h_t[:],
                        evsT_t[:, :w], start=True, stop=True,
                    )
                    nc.scalar.activation(xdT_t[:, g0:g0 + w], psB[:, :w], AF.Copy)

            # xinT arrives during early C (first use: E of group 0)
            nc.sync.dma_start(xinT_t[:], xinT_d[:])

            # ---------------- phases C-E per 512-row group
            SCR = 64       # de-interleave overrun scratch columns
            SEL_LA = 4     # selg prefetch lookahead (groups)
            mxagg = max(T * R for (dh, R, T, _, _) in geo)
            mxsel = max(4 * T * R for (dh, R, T, _, _) in geo)
            with (
                tc.tile_pool(name="pS", bufs=SEL_LA + 2) as pS,
                tc.tile_pool(name="pG", bufs=2) as pG,
                tc.tile_pool(name="psAG", bufs=2, space="PSUM") as psAG_pool,
                tc.tile_pool(name="psGXY", bufs=1, space="PSUM") as psGXY_pool,
                tc.tile_pool(name="psBXY", bufs=1, space="PSUM") as psBXY_pool,
                tc.tile_pool(name="psH", bufs=2, space="PSUM") as psH_pool,
            ):
                groups = list(range(0, NBLK, GRP))
                sel_tiles = {}

                def sel_fetch(g):
                    nb = min(GRP, NBLK - g)
                    sel0 = geo[g][4]
                    sel1 = (geo[g + nb][4] if g + nb < NBLK else SELTOT)
                    selg = pS.tile([128, GRP * mxsel], f8, tag="selg")
                    nc.sync.dma_start(selg[:, :sel1 - sel0],
                                      selxy_d[:, sel0:sel1])
                    sel_tiles[g] = selg

                for g in groups[:SEL_LA]:
                    sel_fetch(g)

                for gi, g in enumerate(groups):
                    if gi + SEL_LA < len(groups):
                        sel_fetch(groups[gi + SEL_LA])
                    nb = min(GRP, NBLK - g)
                    gw = nb * 128
                    g0 = g * 128
                    sel0 = geo[g][4]
                    selg = sel_tiles.pop(g)
                    agXY_sb = pG.tile([K, 2, GRP * 128 + SCR], f16, tag="agxy")
                    for q in range(nb):
                        s = g + q
                        dh, R, T, toff, soff = geo[s]
                        so = soff - sel0
                        # strided matmul out: X cols land in [:, 0, jR:...],
                        # Y cols in [:, 1, jR:...] -> row-contiguous halves
                        # with no de-interleave pass.
                        agXY = psAG_pool.tile([K, 2, mxagg], f32, tag="agXY")
                        for j in range(T):
                            nc.tensor.matmul(
                                agXY[:, :, j * R:(j + 1) * R],
                                evg_tile(toff + j),
                                selg[:, so + 4 * j * R:so + 4 * (j + 1) * R]
                                .rearrange("k (h x) -> k h x", h=2),
                                start=True, stop=True, perf_mode=DR,
                            )
                        if q % 2 == 0:
                            nc.vector.tensor_copy(
                                agXY_sb[:, :, q * 128:q * 128 + T * R],
                                agXY[:, :, :T * R])
                        else:
                            nc.scalar.copy(
                                agXY_sb[:, :, q * 128:q * 128 + T * R],
                                agXY[:, :, :T * R])

                    # C2: gx^T, gy^T
                    psGXY = psGXY_pool.tile([P, 2, GRP * 128], f32, tag="psGXY")
                    nc.tensor.matmul(psGXY[:, 0, :gw], s2h_t[:],
                                     agXY_sb[:, 0, :gw], start=True, stop=True)
                    nc.tensor.matmul(psGXY[:, 1, :gw], s2h_t[:],
                                     agXY_sb[:, 1, :gw], start=True, stop=True)
                    gxy_sb = pG.tile([P, 2, GRP * 128], f16, tag="gxy")
                    nc.vector.tensor_copy(gxy_sb[:, 0, :gw], psGXY[:, 0, :gw])
                    nc.scalar.copy(gxy_sb[:, 1, :gw], psGXY[:, 1, :gw])

                    # D: xg = tanh(gx*(B_re gx) + gy*(B_im gy))
                    psBXY = psBXY_pool.tile([P, 2, GRP * 128], f32, tag="psBXY")
                    nc.tensor.matmul(psBXY[:, 0, :gw], bret_t[:],
                                     gxy_sb[:, 0, :gw], start=True, stop=True)
                    nc.tensor.matmul(psBXY[:, 1, :gw], bimt_t[:],
                                     gxy_sb[:, 1, :gw], start=True, stop=True)
                    t1 = pG.tile([P, 2, GRP * 128], f32, tag="t1")
                    nc.vector.tensor_mul(t1[:, :, :gw], gxy_sb[:, :, :gw],
                                         psBXY[:, :, :gw])
                    t2 = pG.tile([P, GRP * 128], f32, tag="t2")
                    nc.vector.tensor_add(t2[:, :gw], t1[:, 0, :gw],
                                         t1[:, 1, :gw])
                    xg_sb = pG.tile([P, GRP * 128], f16, tag="xg")
                    nc.scalar.activation(xg_sb[:, :gw], t2[:, :gw], AF.Tanh)

                    # E: MLP + residual
                    psH1 = psH_pool.tile([P, GRP * 128], f32, tag="psH")
                    nc.tensor.matmul(psH1[:, :gw], w1t_t[:, 0, :],
                                     xinT_t[:, g0:g0 + gw],
                                     start=True, stop=False)
                    nc.tensor.matmul(psH1[:, :gw], w1t_t[:, 1, :],
                                     xdT_t[:, g0:g0 + gw],
                                     start=False, stop=False)
                    nc.tensor.matmul(psH1[:, :gw], w1t_t[:, 2, :],
                                     xg_sb[:, :gw], start=False, stop=True)
                    h_sb = pG.tile([P, GRP * 128], f16, tag="h")
                    nc.scalar.activation(h_sb[:, :gw], psH1[:, :gw], AF.Relu,
                                         bias=b1_t[:])
                    psH2 = psH_pool.tile([P, GRP * 128], f32, tag="psH")
                    nc.tensor.matmul(psH2[:, :gw], w2t_t[:],
                                     h_sb[:, :gw], start=True, stop=True)
                    h2_sb = pG.tile([P, GRP * 128], f16, tag="h")
                    nc.scalar.activation(h2_sb[:, :gw], psH2[:, :gw], AF.Relu,
                                         bias=b2_t[:])
                    psH3 = psH_pool.tile([P, GRP * 128], f32, tag="psH")
                    nc.tensor.matmul(psH3[:, :gw], w3t_t[:],
                                     h2_sb[:, :gw], start=True, stop=True)
                    out_sb = pG.tile([P, GRP * 128], f16, tag="out")
                    nc.vector.scalar_tensor_tensor(
                        out_sb[:, :gw], psH3[:, :gw], b3_t[:],
                        xinT_t[:, g0:g0 + gw],
                        op0=mybir.AluOpType.add, op1=mybir.AluOpType.add)
                    nc.sync.dma_start(outT_d[:, g0:g0 + gw], out_sb[:, :gw])

    return nc


# ---------------------------------------------------------------- top level

_CACHE = {}


def _get_bass(meta):
    key = tuple(meta["d_slots"].tolist())
    if key not in _CACHE:
        _CACHE[key] = build_bass(meta)
    return _CACHE[key]


def kernel(_trace=False, **inputs):
    in_maps, core_perm, meta = build_host_data(inputs)
    nc = _get_bass(meta)
    res = bass_utils.run_bass_kernel_spmd(
        nc, in_maps, core_ids=list(range(NCORES)), trace=_trace,
        trace_cores=list(range(NCORES)) if _trace else None,
    )
    out = np.zeros((B, N, P), np.float32)
    for c in range(NCORES):
        b = c // 2
        perm = core_perm[c]
        valid = perm >= 0
        outT = res.results[c]["outT"]           # [P, ROWS]
        out[b, perm[valid]] = np.asarray(outT, np.float32).T[valid]
    if _trace:
        return out, res
    return out


# revision 21
# speedup vs baseline: 1.0034x; 1.0034x over previous
"""DiffusionNetBlock on 8 Trainium2 NeuronCores.

Strategy (data-parallel over batch x row-halves, 8 cores = 4 batches x 2):
  core c = 2*b + h owns batch b and half of its mesh vertices.

Host-side prep (sharding/layout only, no model math beyond input folding):
  - fold vertex_areas into x_in, precompute the spectral heat scale
    exp(-evals x times) (tiny [K,P] per batch), transpose weights.
  - the sparse gradient (COO, E=160k edges/batch) is laid out for the
    device: rows of each batch are degree-sorted into 128-row blocks,
    blocks dealt to the two cores, and each block padded to a fixed
    per-slot degree D (equalized across cores so one NEFF serves all 8).
    Edges become dense fp8 streams xev = evecs[col] tiled as DoubleRow
    pairs [128 edges, 2 k-tiles, K]; the weighted segment-sum over rows
    is a 256-deep fp8 DoubleRow matmul with a selector carrying the
    gradX/gradY values, fully on the PE with f32 PSUM accumulation.

Device kernel (Bass/Tile, same program on all 8 cores):
  A: x_spec = evecs^T @ (a*x_in)          (PSUM accum over 157 chunks)
     s2 = exp(-lam t) * x_spec            (one DVE op)
  B: x_diffuse^T = s2^T @ evecs^T         (kept in SBUF, [P, rows])
  C: agX^T/agY^T per 128-row block via fp8 DoubleRow selector matmuls
     gx^T = s2^T @ agX^T, gy^T = s2^T @ agY^T
  D: xg = tanh(gx*(B_re gx) + gy*(B_im gy))
  E: 3-layer MLP on [x_in; x_diffuse; xg], + residual
  All of C-E runs in transposed [feature, row] layout in 512-row groups.
Host inverse-permutes/transposes the output.
"""

import math
import os
import sys

import ml_dtypes
import numpy as np

sys.path.insert(0, "/opt/trn_rl_repo")

from concourse import bass, mybir  # noqa: E402
from concourse import bass_utils  # noqa: E402
from concourse.tile import TileContext  # noqa: E402
from concourse.vector_clock import ScopedClock, VectorClock  # noqa: E402

B, N, P, K, E = 4, 20000, 128, 128, 160000
NCORES = 8
NBLK = 79                    # 128-row blocks per core
ROWS = NBLK * 128            # 10112 row slots per core
TOTBLK = 2 * NBLK            # 158 blocks per batch (20224 >= 20000 row slots)
GRP = 4                      # blocks per 512-wide processing group
NCHUNK = (N + 127) // 128    # 157 n-chunks for phase A (20096 padded)
NPAD = NCHUNK * 128
NCHUNK_H = (NCHUNK + 1) // 2  # chunks per core: pair-split + AllReduce
HOIST = 4                    # C1 groups emitted before phase A (fill PE)

f32 = mybir.dt.float32
f32r = mybir.dt.float32r
f16 = mybir.dt.float16
f8 = mybir.dt.float8e4
f8np = ml_dtypes.float8_e4m3


# --------------------------------------------------------------- BIR fixup
# This toolchain's walrus encodes at most ONE sync wait per instruction
# ("Too many sync wait commands"), but Tile's add_semaphores freely
# attaches several. Hoist excess waits onto EventSemaphore carriers on
# the same engine, inserted just before the over-subscribed instruction.

def _split_excess_waits(bir_json: bytes) -> bytes:
    import json
    d = json.loads(bir_json)
    n_split = 0
    for fn in d.get("functions", []):
        for blk in fn.get("blocks", []):
            insts = blk.get("instructions")
            if not insts:
                continue
            out = []
            changed = False
            for ins in insts:
                si = ins.get("sync_info") or {}
                ow = si.get("on_wait") or []
                if len(ow) > 1 and "engine" in ins:
                    for w in ow[:-1]:
                        n_split += 1
                        out.append({
                            "debug": ins.get("debug", 0),
                            "engine": ins["engine"],
                            "ins": [],
                            "outs": [],
                            "name": f"{ins['name']}-xw{n_split}",
                            "opcode": "EventSemaphore",
                            "sync_info": {"on_update": [], "on_wait": [w]},
                        })
                    si["on_wait"] = [ow[-1]]
                    changed = True
                out.append(ins)
            if changed:
                blk["instructions"] = out
    if n_split == 0:
        return bir_json
    return json.dumps(d).encode()


_orig_compile_bir_kernel = bass_utils.compile_bir_kernel


def _patched_compile_bir_kernel(bir_json, tmpdir, neff_name="file.neff"):
    return _orig_compile_bir_kernel(_split_excess_waits(bir_json), tmpdir,
                                    neff_name)


def _install_birfix():
    from concourse import bass2jax
    if bass_utils.compile_bir_kernel.__name__ != "_patched_compile_bir_kernel":
        bass_utils.compile_bir_kernel = _patched_compile_bir_kernel
    if bass2jax.compile_bir_kernel.__name__ != "_patched_compile_bir_kernel":
        bass2jax.compile_bir_kernel = _patched_compile_bir_kernel


_install_birfix()


class FixedTileContext(TileContext):
    """Stock _drain_and_barrier stuffs every outstanding sem wait onto one
    SP Drain; TRN2 TPB_CTRL encoding only fits 1-2 sync waits and walrus
    dies with "Too many sync wait commands". Split the final global-clock
    wait into one Drain per logical proc."""

    def _drain_and_barrier(self, tick_clock, wait_clock):
        gc = tick_clock.global_clock
        n = len(gc)
        for p in range(n):
            if gc[p] > 0:
                vec = [0] * n
                vec[p] = gc[p]
                w = self.nc.sync.drain()
                wait_clock.add_sem_waits(w.ins, ScopedClock({None: VectorClock(vec)}))
        # The per-proc drains above run serially on SP, so every wait is
        # already satisfied here; emit the final drain bare.
        self.nc.sync.drain()
        self.nc.all_engine_barrier()
        assert self.sems is not None
        popped = self.nc._tile_sem_poison_stack.pop()
        assert popped is self._sem_poison
        self.nc.clear_and_free_semaphores(list(self.sems.allocated().values()))
        self.nc.all_engine_barrier()


# ---------------------------------------------------------------- host prep


def _plan_slots(grad_rows):
    """Degree-sort rows per batch into blocks, deal to cores, and compute
    the global per-slot degree D (equalized across all 8 cores)."""
    perms = []          # per batch: [TOTBLK*128] row ids (-1 = pad)
    degs = []
    d_blocks = np.zeros((B, 2, NBLK), np.int64)
    for b in range(B):
        deg = np.bincount(np.asarray(grad_rows[b]), minlength=N)
        order = np.argsort(-deg, kind="stable")
        perm = np.concatenate([order, np.full(TOTBLK * 128 - N, -1, np.int64)])
        dblk = deg[np.maximum(perm, 0)] * (perm >= 0)
        dblk = dblk.reshape(TOTBLK, 128).max(axis=1)
        for i in range(TOTBLK):
            d_blocks[b, i % 2, i // 2] = dblk[i]
        perms.append(perm)
        degs.append(deg)
    d_slots = np.maximum(d_blocks.max(axis=(0, 1)), 1)   # [NBLK]
    assert d_slots.max() <= 128, d_slots.max()
    return perms, degs, d_slots


def _slot_geometry(d_slots):
    """fp8 DoubleRow geometry. Per slot: half-degree dh (edges per row per
    k-tile), rows-per-instruction R, instructions T, stream tile offset,
    and the column offset of this slot's selector block (4*R*T cols: per
    tile, per k-tile half, [selX_R | selY_R])."""
    geo = []
    t_off = 0
    s_off = 0
    for D in d_slots.tolist():
        dh = (D + 1) // 2
        R = 128 // dh
        T = math.ceil(128 / R)
        geo.append((dh, R, T, t_off, s_off))
        t_off += T
        s_off += 4 * T * R
    return geo, t_off, s_off


def build_host_data(inputs):
    x_in = np.asarray(inputs["x_in"], np.float32)
    areas = np.asarray(inputs["vertex_areas"], np.float32)
    evals = np.asarray(inputs["evals"], np.float32)
    evecs = np.asarray(inputs["evecs"], np.float32)
    gxv = np.asarray(inputs["gradX_vals"], np.float32)
    gyv = np.asarray(inputs["gradY_vals"], np.float32)
    grows = np.asarray(inputs["grad_rows"], np.int64)
    gcols = np.asarray(inputs["grad_cols"], np.int64)
    times = np.clip(np.asarray(inputs["diffusion_times"], np.float32), 1e-8, None)
    W1 = np.asarray(inputs["W1"], np.float32)
    b1 = np.asarray(inputs["b1"], np.float32)
    W2 = np.asarray(inputs["W2"], np.float32)
    b2 = np.asarray(inputs["b2"], np.float32)
    W3 = np.asarray(inputs["W3"], np.float32)
    b3 = np.asarray(inputs["b3"], np.float32)
    B_re = np.asarray(inputs["B_re"], np.float32)
    B_im = np.asarray(inputs["B_im"], np.float32)

    perms, degs, d_slots = _plan_slots(grows)
    geo, TT, SELTOT = _slot_geometry(d_slots)

    # phase A inputs, partition-major: ax[p, c, 0, :] = evecs row c*128+p,
    # ax[p, c, 1, :] = (a*x_in) row c*128+p
    ax_all = np.zeros((B, NPAD, 2, P), np.float16)
    ax_all[:, :N, 0, :] = evecs.astype(np.float16)
    ax_all[:, :N, 1, :] = (x_in * areas[:, :, None]).astype(np.float16)
    ax_all = np.ascontiguousarray(
        ax_all.reshape(B, NCHUNK, 128, 2, P).transpose(0, 2, 1, 3, 4))

    in_maps = []
    core_perm = []
    for b in range(B):
        rows_b, cols_b = grows[b], gcols[b]
        esort = np.argsort(rows_b, kind="stable")
        deg = degs[b]
        rowptr = np.zeros(N + 1, np.int64)
        rowptr[1:] = np.cumsum(deg)
        scale = np.exp(-evals[b][:, None] * times[None, :]).astype(np.float32)
        ev8 = evecs[b].astype(f8np)
        for h in range(2):
            blk_ids = 2 * np.arange(NBLK) + h          # block index within batch
            perm_own = perms[b].reshape(TOTBLK, 128)[blk_ids].reshape(-1)  # [ROWS]
            core_perm.append(perm_own)
            pv = np.maximum(perm_own, 0)
            valid = perm_own >= 0

            # per-row padded edge grid, slot by slot, split into 2 k-tile
            # halves for fp8 DoubleRow (256-edge contraction / instruction)
            col_stream = np.zeros((TT, 2, 128), np.int64)
            selxy = np.zeros((128, SELTOT), f8np)
            for s, (dh, R, T, toff, soff) in enumerate(geo):
                rows_blk = perm_own[s * 128:(s + 1) * 128]
                rb = np.maximum(rows_blk, 0)
                cnt = np.where(rows_blk >= 0, deg[rb], 0)
                c0 = (cnt + 1) // 2                      # ceil half
                starts = [np.zeros_like(c0), c0]
                halves = [c0, cnt - c0]
                G = T * R
                e = np.arange(128)
                ei, ed = e // dh, e % dh             # row-in-tile, slot-in-row
                emask = ei < R
                eis = np.where(emask, ei, 0)
                sel_t = np.zeros((T, 2, 128, 2 * R), np.float32)
                for hh in range(2):
                    idx = (rowptr[rb][:, None] + starts[hh][:, None]
                           + np.arange(dh)[None, :])
                    mask = np.arange(dh)[None, :] < halves[hh][:, None]
                    eid = esort[np.where(mask, idx, 0)]
                    cm = np.where(mask, cols_b[eid], 0)          # [128, dh]
                    vxm = np.where(mask, gxv[b][eid], 0.0)
                    vym = np.where(mask, gyv[b][eid], 0.0)
                    pad = ((0, G - 128), (0, 0))
                    cmp_ = np.pad(cm, pad).reshape(T, R * dh)
                    col_stream[toff:toff + T, hh] = np.pad(
                        cmp_, ((0, 0), (0, 128 - R * dh)))
                    vxm = np.pad(vxm, pad).reshape(T, R, dh)
                    vym = np.pad(vym, pad).reshape(T, R, dh)
                    sel_t[:, hh, e, eis] = vxm[:, eis, ed] * emask
                    sel_t[:, hh, e, R + eis] = vym[:, eis, ed] * emask
                selxy[:, soff:soff + 4 * T * R] = (
                    sel_t.transpose(2, 0, 1, 3).reshape(128, T * 4 * R)
                    .astype(f8np))

            evg = np.ascontiguousarray(
                ev8[col_stream].transpose(2, 0, 1, 3))   # [128, TT, 2, K]

            in_maps.append({
                "evg": evg,
                "selxy": selxy,
                "ax": ax_all[b],
                "evsT": np.ascontiguousarray(
                    (evecs[b][pv].T * valid[None, :]).astype(np.float16)),
                "xinT": np.ascontiguousarray(
                    (x_in[b][pv].T * valid[None, :]).astype(np.float16)),
                "scale": scale,
                "w1t": np.ascontiguousarray(W1.T.reshape(3, P, P).astype(np.float16)),
                "w2t": np.ascontiguousarray(W2.T.astype(np.float16)),
                "w3t": np.ascontiguousarray(W3.T.astype(np.float16)),
                "bret": np.ascontiguousarray(B_re.T.astype(np.float16)),
                "bimt": np.ascontiguousarray(B_im.T.astype(np.float16)),
                "b1": b1.reshape(P, 1).copy(),
                "b2": b2.reshape(P, 1).copy(),
                "b3": b3.reshape(P, 1).copy(),
            })

    meta = {"geo": geo, "TT": TT, "SELTOT": SELTOT, "d_slots": d_slots}
    return in_maps, core_perm, meta


# ------------------------------------------------------------ device kernel


def build_bass(meta):
    geo = meta["geo"]
    TT = meta["TT"]
    SELTOT = meta["SELTOT"]

    nc = bass.Bass("TRN2", target_bir_lowering=False, debug=False,
                   num_devices=NCORES)

    evg_d = nc.dram_tensor("evg", [128, TT, 2, K], f8, kind="ExternalInput")
    selxy_d = nc.dram_tensor("selxy", [128, SELTOT], f8, kind="ExternalInput")
    ax_d = nc.dram_tensor("ax", [128, NCHUNK, 2, P], f16,
                          kind="ExternalInput")
    evsT_d = nc.dram_tensor("evsT", [K, ROWS], f16, kind="ExternalInput")
    xinT_d = nc.dram_tensor("xinT", [P, ROWS], f16, kind="ExternalInput")
    scale_d = nc.dram_tensor("scale", [K, P], f32, kind="ExternalInput")
    w1t_d = nc.dram_tensor("w1t", [3, P, P], f16, kind="ExternalInput")
    w2t_d = nc.dram_tensor("w2t", [P, P], f16, kind="ExternalInput")
    w3t_d = nc.dram_tensor("w3t", [P, P], f16, kind="ExternalInput")
    bret_d = nc.dram_tensor("bret", [P, P], f16, kind="ExternalInput")
    bimt_d = nc.dram_tensor("bimt", [P, P], f16, kind="ExternalInput")
    b1_d = nc.dram_tensor("b1", [P, 1], f32, kind="ExternalInput")
    b2_d = nc.dram_tensor("b2", [P, 1], f32, kind="ExternalInput")
    b3_d = nc.dram_tensor("b3", [P, 1], f32, kind="ExternalInput")
    outT_d = nc.dram_tensor("outT", [P, ROWS], f16, kind="ExternalOutput")

    AF = mybir.ActivationFunctionType
    DR = mybir.MatmulPerfMode.DoubleRow
    XCH = 16       # evg DoubleRow tiles per DMA chunk
    EVG_BUFS = 8   # evg ring depth (chunks)
    SCR = 64       # de-interleave overrun scratch columns
    SEL_LA = 4     # selg prefetch lookahead (groups)
    mxagg = max(T * R for (dh, R, T, _, _) in geo)
    mxsel = max(4 * T * R for (dh, R, T, _, _) in geo)
    groups = list(range(0, NBLK, GRP))

    with FixedTileContext(nc) as tc:
        with (
            tc.tile_pool(name="consts", bufs=1) as cpool,
            tc.tile_pool(name="xdpool", bufs=1) as xdpool,
            tc.tile_pool(name="pX", bufs=EVG_BUFS) as pX,
            tc.tile_pool(name="pS", bufs=HOIST + 6) as pS,
            tc.tile_pool(name="pG", bufs=2) as pG,
            tc.tile_pool(name="pGH", bufs=HOIST + 1) as pGH,
            tc.tile_pool(name="psAG", bufs=2, space="PSUM") as psAG_pool,
        ):
            scale_t = cpool.tile([K, P], f32, tag="scale")
            nc.scalar.dma_start(scale_t[:], scale_d[:])
            wh = cpool.tile([P, 7, P], f16, tag="wh")
            nc.scalar.dma_start(wh[:, 0:3, :], w1t_d[:].rearrange("s p q -> p s q"))
            nc.scalar.dma_start(wh[:, 3, :], w2t_d[:])
            nc.scalar.dma_start(wh[:, 4, :], w3t_d[:])
            nc.scalar.dma_start(wh[:, 5, :], bret_d[:])
            nc.scalar.dma_start(wh[:, 6, :], bimt_d[:])
            w1t_t = wh[:, 0:3, :]
            w2t_t = wh[:, 3, :]
            w3t_t = wh[:, 4, :]
            bret_t = wh[:, 5, :]
            bimt_t = wh[:, 6, :]
            b1_t = cpool.tile([P, 1], f32, tag="b1")
            nc.scalar.dma_start(b1_t[:], b1_d[:])
            b2_t = cpool.tile([P, 1], f32, tag="b2")
            nc.scalar.dma_start(b2_t[:], b2_d[:])
            b3_t = cpool.tile([P, 1], f32, tag="b3")
            nc.scalar.dma_start(b3_t[:], b3_d[:])
            s2_t = cpool.tile([K, P], f32, tag="s2")
            s2h_t = cpool.tile([K, P], f16, tag="s2h")
            xdT_t = xdpool.tile([P, ROWS], f16, tag="xdT")
            xinT_t = xdpool.tile([P, ROWS], f16, tag="xinT")

            # eager evg prefetch: the whole stream is an input, so issue
            # every chunk DMA up front on gpsimd (which carries nothing
            # else — the ring's WAR waits stall only this queue) and let
            # the pool's ring recycling pace it against consumption.
            evg_chunks = []
            for t0 in range(0, TT, XCH):
                w = min(XCH, TT - t0)
                xt = pX.tile([128, XCH, 2, K], f8, tag="evg")
                nc.gpsimd.dma_start(xt[:, :w], evg_d[:, t0:t0 + w])
                evg_chunks.append(xt)

            def evg_tile(t):
                return evg_chunks[t // XCH][:, t % XCH, :, :]

            sel_tiles = {}

            def sel_fetch(g):
                nb = min(GRP, NBLK - g)
                sel0 = geo[g][4]
                sel1 = (geo[g + nb][4] if g + nb < NBLK else SELTOT)
                selg = pS.tile([128, GRP * mxsel], f8, tag="selg")
                nc.sync.dma_start(selg[:, :sel1 - sel0],
                                  selxy_d[:, sel0:sel1])
                sel_tiles[g] = selg

            def c1_group(g, pool):
                """Selector matmuls + PSUM->SBUF copy for one 512-row group;
                returns the group's agXY tile [K, 2, 512+SCR] f16."""
                nb = min(GRP, NBLK - g)
                sel0 = geo[g][4]
                selg = sel_tiles.pop(g)
                agXY_sb = pool.tile([K, 2, GRP * 128 + SCR], f16, tag="agxy")
                for q in range(nb):
                    dh, R, T, toff, soff = geo[g + q]
                    so = soff - sel0
                    # strided matmul out: X cols land in [:, 0, jR:...],
                    # Y cols in [:, 1, jR:...] -> row-contiguous halves
                    # with no de-interleave pass.
                    agXY = psAG_pool.tile([K, 2, mxagg], f32, tag="agXY")
                    for j in range(T):
                        nc.tensor.matmul(
                            agXY[:, :, j * R:(j + 1) * R],
                            evg_tile(toff + j),
                            selg[:, so + 4 * j * R:so + 4 * (j + 1) * R]
                            .rearrange("k (h x) -> k h x", h=2),
                            start=True, stop=True, perf_mode=DR,
                        )
                    if q % 2 == 0:
                        nc.vector.tensor_copy(
                            agXY_sb[:, :, q * 128:q * 128 + T * R],
                            agXY[:, :, :T * R])
                    else:
                        nc.scalar.copy(
                            agXY_sb[:, :, q * 128:q * 128 + T * R],
                            agXY[:, :, :T * R])
                return agXY_sb

            # ---------------- hoisted C1: fill the PE while ax streams in
            for g in groups[:HOIST]:
                sel_fetch(g)
            hoisted = {}
            for gi in range(HOIST):
                hoisted[gi] = c1_group(groups[gi], pGH)

            # ---------------- phase A: x_spec, s2
            ACH = 8
            with (
                tc.tile_pool(name="pA", bufs=6) as pA,
                tc.tile_pool(name="psA", bufs=1, space="PSUM") as psA_pool,
            ):
                psA = psA_pool.tile([K, P], f32, tag="psA")
                for c0 in range(0, NCHUNK, ACH):
                    w = min(ACH, NCHUNK - c0)
                    ax_t = pA.tile([128, ACH, 2, P], f16, tag="axA")
                    nc.sync.dma_start(ax_t[:, :w], ax_d[:, c0:c0 + w])
                    for i in range(w):
                        nc.tensor.matmul(
                            psA[:], ax_t[:, i, 0, :], ax_t[:, i, 1, :],
                            start=(c0 + i == 0),
                            stop=(c0 + i == NCHUNK - 1),
                        )
                nc.vector.tensor_mul(s2_t[:], scale_t[:], psA[:])
                nc.vector.tensor_copy(s2h_t[:], s2_t[:])
            # evsT + xinT prefetch during late phase A / B
            evsT_t = xdpool.tile([K, ROWS], f16, tag="evsT")
            nc.sync.dma_start(evsT_t[:], evsT_d[:])
            nc.sync.dma_start(xinT_t[:], xinT_d[:])

            # ---------------- phase B: x_diffuse^T resident in SBUF
            with tc.tile_pool(name="psB", bufs=2, space="PSUM") as psB_pool:
                for g0 in range(0, ROWS, 512):
                    w = min(512, ROWS - g0)
                    psB = psB_pool.tile([P, 512], f32, tag="psB")
                    nc.tensor.matmul(
                        psB[:, :w], s2h_t[:],
                        evsT_t[:, g0:g0 + w], start=True, stop=True,
                    )
                    nc.scalar.activation(xdT_t[:, g0:g0 + w], psB[:, :w], AF.Copy)

            # ---------------- phases C-E per 512-row group
            with (
                tc.tile_pool(name="psGXY", bufs=1, space="PSUM") as psGXY_pool,
                tc.tile_pool(name="psBXY", bufs=1, space="PSUM") as psBXY_pool,
                tc.tile_pool(name="psH", bufs=2, space="PSUM") as psH_pool,
            ):
                next_fetch = HOIST

                for gi, g in enumerate(groups):
                    while (next_fetch < len(groups)
                           and next_fetch <= gi + SEL_LA):
                        sel_fetch(groups[next_fetch])
                        next_fetch += 1
                    nb = min(GRP, NBLK - g)
                    gw = nb * 128
                    g0 = g * 128
                    if gi in hoisted:
                        agXY_sb = hoisted.pop(gi)
                    else:
                        agXY_sb = c1_group(g, pG)

                    # C2: gx^T, gy^T
                    psGXY = psGXY_pool.tile([P, 2, GRP * 128], f32, tag="psGXY")
                    nc.tensor.matmul(psGXY[:, 0, :gw], s2h_t[:],
                                     agXY_sb[:, 0, :gw], start=True, stop=True)
                    nc.tensor.matmul(psGXY[:, 1, :gw], s2h_t[:],
                                     agXY_sb[:, 1, :gw], start=True, stop=True)
                    gxy_sb = pG.tile([P, 2, GRP * 128], f16, tag="gxy")
                    nc.vector.tensor_copy(gxy_sb[:, 0, :gw], psGXY[:, 0, :gw])
                    nc.scalar.copy(gxy_sb[:, 1, :gw], psGXY[:, 1, :gw])

                    # D: xg = tanh(gx*(B_re gx) + gy*(B_im gy))
                    psBXY = psBXY_pool.tile([P, 2, GRP * 128], f32, tag="psBXY")
                    nc.tensor.matmul(psBXY[:, 0, :gw], bret_t[:],
                                     gxy_sb[:, 0, :gw], start=True, stop=True)
                    nc.tensor.matmul(psBXY[:, 1, :gw], bimt_t[:],
                                     gxy_sb[:, 1, :gw], start=True, stop=True)
                    t1 = pG.tile([P, 2, GRP * 128], f32, tag="t1")
                    nc.vector.tensor_mul(t1[:, :, :gw], gxy_sb[:, :, :gw],
                                         psBXY[:, :, :gw])
                    t2 = pG.tile([P, GRP * 128], f32, tag="t2")
                    nc.vector.tensor_add(t2[:, :gw], t1[:, 0, :gw],
                                         t1[:, 1, :gw])
                    xg_sb = pG.tile([P, GRP * 128], f16, tag="xg")
                    nc.scalar.activation(xg_sb[:, :gw], t2[:, :gw], AF.Tanh)

                    # E: MLP + residual
                    psH1 = psH_pool.tile([P, GRP * 128], f32, tag="psH")
                    nc.tensor.matmul(psH1[:, :gw], w1t_t[:, 0, :],
                                     xinT_t[:, g0:g0 + gw],
                                     start=True, stop=False)
                    nc.tensor.matmul(psH1[:, :gw], w1t_t[:, 1, :],
                                     xdT_t[:, g0:g0 + gw],
                                     start=False, stop=False)
                    nc.tensor.matmul(psH1[:, :gw], w1t_t[:, 2, :],
                                     xg_sb[:, :gw], start=False, stop=True)
                    h_sb = pG.tile([P, GRP * 128], f16, tag="h")
                    nc.scalar.activation(h_sb[:, :gw], psH1[:, :gw], AF.Relu,
                                         bias=b1_t[:])
                    psH2 = psH_pool.tile([P, GRP * 128], f32, tag="psH")
                    nc.tensor.matmul(psH2[:, :gw], w2t_t[:],
                                     h_sb[:, :gw], start=True, stop=True)
                    h2_sb = pG.tile([P, GRP * 128], f16, tag="h")
                    nc.scalar.activation(h2_sb[:, :gw], psH2[:, :gw], AF.Relu,
                                         bias=b2_t[:])
                    psH3 = psH_pool.tile([P, GRP * 128], f32, tag="psH")
                    nc.tensor.matmul(psH3[:, :gw], w3t_t[:],
                                     h2_sb[:, :gw], start=True, stop=True)
                    out_sb = pG.tile([P, GRP * 128], f16, tag="out")
                    nc.vector.scalar_tensor_tensor(
                        out_sb[:, :gw], psH3[:, :gw], b3_t[:],
                        xinT_t[:, g0:g0 + gw],
                        op0=mybir.AluOpType.add, op1=mybir.AluOpType.add)
                    nc.sync.dma_start(outT_d[:, g0:g0 + gw], out_sb[:, :gw])

    return nc


# ---------------------------------------------------------------- top level

_CACHE = {}


def _get_bass(meta):
    key = tuple(meta["d_slots"].tolist())
    if key not in _CACHE:
        _CACHE[key] = build_bass(meta)
    return _CACHE[key]


def kernel(_trace=False, **inputs):
    in_maps, core_perm, meta = build_host_data(inputs)
    nc = _get_bass(meta)
    res = bass_utils.run_bass_kernel_spmd(
        nc, in_maps, core_ids=list(range(NCORES)), trace=_trace,
        trace_cores=list(range(NCORES)) if _trace else None,
    )
    out = np.zeros((B, N, P), np.float32)
    for c in range(NCORES):
        b = c // 2
        perm = core_perm[c]
        valid = perm >= 0
        outT = res.results[c]["outT"]           # [P, ROWS]
        out[b, perm[valid]] = np.asarray(outT, np.float32).T[valid]
    if _trace:
        return out, res
    return out
